# revision 17
# baseline (speedup 1.0000x reference)
"""Trainium2 Bass kernel for nn_Decay2D (decay-masked linear attention).

Math: the reference's Hillis-Steele scan with decay-squaring order composes
to coefficient d^ceil((t-s)/2) on store[s] = scale*k_s v_s^T, so

    out[t] = scale^2 * sum_{s<=t} d^ceil((t-s)/2) (q_t . k_s) v_s  @ Wo^T

computed as chunked linear attention with two [K,V] carry states (even/odd
decay chains), never materializing the [B,T,K,V] memory.

Sharding: 8 cores = 4 batches x 2 sequence halves. Each core builds the
carry state over its (truncated) prefix rows and runs full attention +
output projection for its own 1024 rows.

Implementation notes (final, ~40.5us HW exec vs 56.6us baseline):
- bf16 on the PE, fp32 PSUM accumulation, fp32 carry states, bf16 output
  (converted back to fp32 on host; adds ~0.03% fro error, gate is 2e-2).
- Biases folded into activation bias= APs (no bias matmuls / ones row);
  k|v transposed jointly (one [128,128] PE transpose per chunk); parity
  decay weights applied as strided tensor_scalar ops on the k columns.
- Dual DMA queues: x loads on the sync HWDGE queue, constants on the
  scalar HWDGE queue (aggregate ~530GB/s); the constant pack is split so
  the projection weights land first and mask constants trail.
- Group-0 projections run per half-group (N=256 matmuls) so the PE
  starts right after the first 0.5MB of x lands.
- Interleaved schedule tuned against the trace: the sequential DVE carry
  chain and mask muls are slotted where PE has independent work, and the
  attention/out-projection pipeline keeps the 3-buffer po PSUM rotation
  fed (out copies split vector/scalar; final chunk DMA split across both
  HWDGE queues).
- gpsimd is avoided for tensor ops (cannot access PSUM, ~10x slower).
"""

from contextlib import ExitStack

import numpy as np

import concourse.bass as bass
import concourse.bacc as bacc
import concourse.mybir as mybir
import concourse.tile as tile
from concourse import bass_utils
from concourse.alu_op_type import AluOpType
from concourse.bass import ts

F32 = mybir.dt.float32
BF16 = mybir.dt.bfloat16
SIG = mybir.ActivationFunctionType.Sigmoid
IDENT = mybir.ActivationFunctionType.Identity

B, T, E, K, V = 4, 2048, 1024, 64, 64
DECAY = 0.9
C = 128          # chunk length
HT = T // 2      # rows per core (sequence half)
NCH = HT // C    # chunks per half (8)
NEC = E // 128   # embed sub-chunks (8)
GW = 512         # group width: 4 chunks per PSUM bank
GCH = GW // C    # chunks per group (4)
NG = HT // GW    # groups per half (2)
DC2 = float(DECAY ** (C // 2))
N_CORES = 8
HW2 = GW // 2    # half-group width
PRE = 128        # truncated prefix length (1 chunk; older rows decay < 2e-3)

# packed-constants layout (single bf16 pack)
def _mklayout(regions):
    out, off = {}, 0
    for n, r, c in regions:
        out[n] = (r, off, c)
        off += c
    return out, off


_HOT, HOT_W = _mklayout([
    ("wkv", 128, NEC * 2 * K), ("wq", 128, NEC * K), ("ident", 128, 128),
    ("mloc4", 128, GW), ("ce4", K, GW), ("co4", K, GW),
])
# f32 sidecar columns: bkv, bq, gamma, wge, wgo
CST_W = 5


def _host_constants():
    d = DECAY
    scale2 = 1.0 - d
    i = np.arange(C)
    j = np.arange(C)
    delta = i[:, None] - j[None, :]
    # intra-chunk decay mask, transposed to [tcol(j=src), trow(i=dst)],
    # scale^2 folded
    mloc = np.where(delta >= 0, d ** np.ceil(delta / 2.0), 0.0) * scale2
    mloc4 = np.tile(np.ascontiguousarray(mloc.T), (1, GCH)).astype(np.float32)
    # boundary coefficient per local row i (scale^2 folded), split by parity
    c = d ** np.ceil((i + 1) / 2.0) * scale2
    ce = np.where(i % 2 == 0, c, 0.0).astype(np.float32)
    co = np.where(i % 2 == 1, c, 0.0).astype(np.float32)
    ce4 = np.tile(np.broadcast_to(ce, (K, C)), (1, GCH)).astype(np.float32)
    co4 = np.tile(np.broadcast_to(co, (K, C)), (1, GCH)).astype(np.float32)
    # state-update row weights (per t within chunk)
    u_o = np.where(j % 2 == 1, d ** ((C - 1 - j) / 2.0), 0.0)
    u_e = np.where(j % 2 == 0, d ** ((C - 2 - j) / 2.0), 0.0)
    wge = (u_o + u_e).astype(np.float32)
    wgo = (u_o + d * u_e).astype(np.float32)
    return {
        "mloc4": mloc4,
        "ce4": np.ascontiguousarray(ce4),
        "co4": np.ascontiguousarray(co4),
        "wge": wge,
        "wgo": wgo,
        "ident": np.eye(128, dtype=np.float32),
    }


def _build_program():
    nc = bacc.Bacc(
        "TRN2",
        debug=False,
        enable_asserts=False,
        target_bir_lowering=False,
        num_devices=N_CORES,
    )

    def din(name, shape, dtype=F32):
        return nc.dram_tensor(name, shape, dtype, kind="ExternalInput").ap()

    chot_d = din("chot", [128, HOT_W], BF16)
    cst_d = din("cst", [128, CST_W], F32)
    xq2 = din("xq2", [NG * 2 * 128, NEC * HW2], BF16)
    xpre2 = din("xpre2", [128, NEC * PRE], BF16)
    woT_d = din("woT", [V, E], BF16)
    out_d = nc.dram_tensor("out", [HT, E], BF16, kind="ExternalOutput").ap()

    with ExitStack() as ctx:
        tc = ctx.enter_context(tile.TileContext(nc))

        consts = ctx.enter_context(tc.tile_pool(name="consts", bufs=1))
        state = ctx.enter_context(tc.tile_pool(name="state", bufs=1))
        xpool = ctx.enter_context(tc.tile_pool(name="xg", bufs=4))
        spool = ctx.enter_context(tc.tile_pool(name="sml", bufs=2))
        opool = ctx.enter_context(tc.tile_pool(name="osb", bufs=6))
        pbig = ctx.enter_context(tc.tile_pool(name="pbig", bufs=3, space="PSUM"))
        pscore = ctx.enter_context(tc.tile_pool(name="pscore", bufs=1, space="PSUM"))
        psml = ctx.enter_context(tc.tile_pool(name="psml", bufs=2, space="PSUM"))
        pattn = ctx.enter_context(tc.tile_pool(name="pattn", bufs=2, space="PSUM"))

        # ---- loads: x on sync queue, constants on scalar queue ----
        def load_xgroup(g, name):
            xg = xpool.tile([128, NEC * GW], BF16, tag="xg", name=name)
            for hh in range(2):
                idx = g * 2 + hh
                nc.sync.dma_start(
                    xg[:, ts(hh, NEC * HW2)],
                    xq2[idx * 128 : (idx + 1) * 128, :],
                )
            return xg

        xg0 = load_xgroup(0, "xg0")
        xp = xpool.tile([128, NEC * PRE], BF16, tag="xp", name="xp", bufs=1)
        nc.sync.dma_start(xp[:], xpre2[:])
        xg1 = load_xgroup(1, "xg1")

        W_SPLIT = _HOT["mloc4"][1]   # weights | masks boundary
        chot = consts.tile([128, HOT_W], BF16, name="chot")
        nc.scalar.dma_start(chot[:, 0:W_SPLIT], chot_d[:, 0:W_SPLIT])
        cst = consts.tile([128, CST_W], F32, name="cst")
        nc.scalar.dma_start(cst[:], cst_d[:])
        wo = consts.tile([V, E], BF16, name="wo")
        nc.scalar.dma_start(wo[:], woT_d[:])
        nc.scalar.dma_start(chot[:, W_SPLIT:], chot_d[:, W_SPLIT:])

        def reg(name):
            r, o, c = _HOT[name]
            return chot[0:r, o : o + c]

        wkv, wq, ident = reg("wkv"), reg("wq"), reg("ident")
        mloc4, ce4, co4 = reg("mloc4"), reg("ce4"), reg("co4")
        bq = cst[0:K, 1:2]
        gamma = cst[:, 2:3]
        wge = cst[:, 3:4]
        wgo = cst[:, 4:5]

        def xg_rhs(xg, ec):
            return xg.rearrange("p (hh a t) -> p a hh t", hh=2, a=NEC)[:, ec]

        qT_all = consts.tile([K, HT], BF16, name="qT_all")
        lt_all = consts.tile([V, HT], BF16, name="lt_all")
        geo_all = state.tile([2 * K, NCH * V], F32, name="geo_all")
        geo_bf = state.tile([2 * K, NCH * V], BF16, name="geo_bf")
        kvTs, knvs, kgeos = {}, {}, {}
        pus, ps_l, sT_l, qTeo_l = {}, {}, {}, {}

        # ---------- building blocks ----------
        def proj_group(g, xg, halves=False):
            pkv = pbig.tile([2 * K, GW], F32, tag="pB", name=f"pkv_g{g}")
            if halves:
                # per-half matmuls: compute starts after the first 0.5MB half
                # of the x group lands instead of the full 1MB
                xv = xg.rearrange("p (hh a t) -> p a hh t", hh=2, a=NEC)
                for hh in range(2):
                    for ec in range(NEC):
                        nc.tensor.matmul(pkv[:, ts(hh, HW2)],
                                         wkv[:, ts(ec, 2 * K)], xv[:, ec, hh],
                                         start=(ec == 0), stop=(ec == NEC - 1))
            else:
                for ec in range(NEC):
                    nc.tensor.matmul(pkv[:], wkv[:, ts(ec, 2 * K)],
                                     xg_rhs(xg, ec),
                                     start=(ec == 0), stop=(ec == NEC - 1))
            kvT = spool.tile([2 * K, GW], BF16, tag=f"kvT_g{g}", name="kvT")
            nc.scalar.activation(kvT[0:K, :], pkv[0:K, :], SIG, bias=cst[0:K, 0:1])
            nc.scalar.activation(kvT[K : 2 * K, :], pkv[K : 2 * K, :], IDENT,
                                 bias=cst[K : 2 * K, 0:1])
            pg = pbig.tile([K, GW], F32, tag="pB", name=f"pq_g{g}")
            for ec in range(NEC):
                nc.tensor.matmul(pg[:], wq[:, ts(ec, K)], xg_rhs(xg, ec),
                                 start=(ec == 0), stop=(ec == NEC - 1))
            nc.scalar.activation(qT_all[:, ts(g, GW)], pg[:], SIG, bias=bq)
            kvTs[g] = kvT

        def trans_group(g):
            kvT = kvTs[g]
            pknv = psml.tile([C, GCH * 2 * K], BF16, tag="pS", name=f"pknv_g{g}")
            for cl in range(GCH):
                nc.tensor.matmul(pknv[:, ts(cl, 2 * K)], kvT[:, ts(cl, C)],
                                 ident[:], is_transpose=True)
            knv = spool.tile([C, GCH * 2 * K], BF16, tag=f"knv_g{g}", name="knv")
            nc.vector.tensor_copy(knv[:], pknv[:])
            kgeo = spool.tile([C, GCH * 2 * K], BF16, tag=f"kg_g{g}", name="kgeo")
            kview = knv.rearrange("p (cl two k) -> p cl two k", cl=GCH, two=2)
            gview = kgeo.rearrange("p (cl two k) -> p cl two k", cl=GCH, two=2)
            nc.vector.tensor_scalar_mul(gview[:, :, 0], kview[:, :, 0], wge)
            nc.vector.tensor_scalar_mul(gview[:, :, 1], kview[:, :, 0], wgo)
            knvs[g], kgeos[g] = knv, kgeo

        def vview(knv, cl):
            return knv[:, cl * 2 * K + K : (cl + 1) * 2 * K]

        def pu_group(g):
            pu = psml.tile([2 * K, GCH * V], F32, tag="pS", name=f"pu_g{g}")
            for cl in range(GCH):
                nc.tensor.matmul(pu[:, ts(cl, V)], kgeos[g][:, ts(cl, 2 * K)],
                                 vview(knvs[g], cl), start=True, stop=True)
            pus[g] = pu

        def chain_seg(g):
            # carry chain states i in [g*GCH+1 .. g*GCH+GCH] using pu_g
            for i in range(g * GCH + 1, g * GCH + GCH + 1):
                if i >= NCH:
                    break
                cl = (i - 1) % GCH
                nc.vector.scalar_tensor_tensor(
                    geo_all[:, ts(i, V)], geo_all[:, ts(i - 1, V)], DC2,
                    pus[g][:, ts(cl, V)], AluOpType.mult, AluOpType.add,
                )
            lo, hi = g * GCH, min(g * GCH + GCH, NCH)
            nc.scalar.copy(geo_bf[:, lo * V : hi * V], geo_all[:, lo * V : hi * V])

        def scores_group(g):
            psc = pscore.tile([C, GW], F32, tag="pSc", name=f"ps_g{g}")
            for cl in range(GCH):
                i = g * GCH + cl
                nc.tensor.matmul(psc[:, ts(cl, C)], kvTs[g][0:K, ts(cl, C)],
                                 qT_all[:, ts(i, C)], start=True, stop=True)
            ps_l[g] = psc

        def masks_group(g):
            sT_b = spool.tile([C, GW], BF16, tag=f"sm{g}", name="sT_b")
            nc.vector.tensor_mul(sT_b[:], ps_l[g][:], mloc4[:])
            sT_l[g] = sT_b
            qTeo = spool.tile([2 * K, GW], BF16, tag=f"qeo{g}", name="qTeo")
            nc.vector.tensor_mul(qTeo[0:K, :], qT_all[:, ts(g, GW)], ce4[:])
            nc.vector.tensor_mul(qTeo[K : 2 * K, :], qT_all[:, ts(g, GW)], co4[:])
            qTeo_l[g] = qTeo

        def attn_chunk(i):
            g, cl = i // GCH, i % GCH
            plt = pattn.tile([V, C], F32, tag="pA", name=f"plt_{i}")
            nc.tensor.matmul(plt[:], vview(knvs[g], cl), sT_l[g][:, ts(cl, C)],
                             start=True, stop=False)
            nc.tensor.matmul(plt[:], geo_bf[:, ts(i, V)], qTeo_l[g][:, ts(cl, C)],
                             start=False, stop=True)
            nc.vector.tensor_copy(lt_all[:, ts(i, C)], plt[:])

        def outproj_chunk(i, split_dma=False):
            out_sb = opool.tile([C, E], BF16, tag="osb", name=f"out_sb_{i}")
            for h in range(2):
                po = pbig.tile([C, GW], F32, tag="pB", name=f"po_{i}_{h}")
                nc.tensor.matmul(po[:], lt_all[:, ts(i, C)], wo[:, ts(h, GW)],
                                 start=True, stop=True)
                if h == 0 and i >= 2:
                    nc.vector.tensor_copy(out_sb[:, ts(h, GW)], po[:])
                else:
                    nc.scalar.copy(out_sb[:, ts(h, GW)], po[:])
                if split_dma:
                    eng = nc.sync if h == 0 else nc.scalar
                    eng.dma_start(out_d[ts(i, C), ts(h, GW)],
                                  out_sb[:, ts(h, GW)])
            if not split_dma:
                nc.sync.dma_start(out_d[ts(i, C), :], out_sb[:])

        # ---------- schedule ----------
        # group 0 projections + transposes
        proj_group(0, xg0, halves=True)
        trans_group(0)

        # prefix phase-1 projection (PE) while scalar runs g0 sigmoids
        pkv1 = pbig.tile([2 * K, PRE], F32, tag="pB", name="pkv1")
        for ec in range(NEC):
            nc.tensor.matmul(pkv1[:], wkv[:, ts(ec, 2 * K)], xp[:, ts(ec, PRE)],
                             start=(ec == 0), stop=(ec == NEC - 1))
        kvT1 = spool.tile([2 * K, PRE], BF16, tag="kvT1", name="kvT1")
        nc.scalar.activation(kvT1[0:K, :], pkv1[0:K, :], SIG, bias=cst[0:K, 0:1])
        nc.scalar.activation(kvT1[K : 2 * K, :], pkv1[K : 2 * K, :], IDENT,
                             bias=cst[K : 2 * K, 0:1])

        # scores g0 (PE busy while kvT1 sigmoid completes)
        scores_group(0)

        # prefix transpose + state
        pknv1 = psml.tile([C, 2 * K], BF16, tag="pS", name="pknv1")
        nc.tensor.matmul(pknv1[:], kvT1[:], ident[:], is_transpose=True)
        knv1 = spool.tile([C, 2 * K], BF16, tag="knv1", name="knv1")
        nc.vector.tensor_copy(knv1[:], pknv1[:])
        kgeo1 = spool.tile([C, 2 * K], BF16, tag="kg1", name="kgeo1")
        nc.vector.tensor_scalar_mul(kgeo1[:, 0:K], knv1[:, 0:K], wge)
        nc.vector.tensor_scalar_mul(kgeo1[:, K : 2 * K], knv1[:, 0:K], wgo)
        pu1 = pattn.tile([2 * K, V], F32, tag="pA", name="pu1")
        nc.tensor.matmul(pu1[:], kgeo1[:], vview(knv1, 0), start=True, stop=True)

        # g0 states + chain segment A (DVE) + masks g0
        pu_group(0)
        nc.vector.tensor_scalar_mul(geo_all[:, 0:V], pu1[:], gamma)
        chain_seg(0)
        masks_group(0)

        # group 1 projections + transposes (PE busy under DVE chain/masks)
        proj_group(1, xg1)
        trans_group(1)

        attn_chunk(0)
        attn_chunk(1)
        pu_group(1)
        chain_seg(1)
        outproj_chunk(0)
        scores_group(1)
        masks_group(1)
        outproj_chunk(1)
        attn_chunk(2)
        outproj_chunk(2)
        attn_chunk(3)
        attn_chunk(4)
        outproj_chunk(3)
        for i in range(5, NCH):
            attn_chunk(i)
            outproj_chunk(i - 1)
        outproj_chunk(NCH - 1, split_dma=True)

    nc.compile()
    return nc


_CACHE = {}


def _get_program():
    if "nc" not in _CACHE:
        _CACHE["nc"] = _build_program()
    return _CACHE["nc"]


def _make_in_maps(x, Wk, bk, Wv, bv, Wq, bq, Wo):
    import ml_dtypes

    bfd = ml_dtypes.bfloat16
    consts = _host_constants()

    def pack2(Wa, Wb):
        # [128, NEC*(outA+outB)]: per embed sub-chunk, [Wa_ec | Wb_ec] columns
        Wab = np.concatenate(
            [Wa.T.reshape(NEC, 128, -1), Wb.T.reshape(NEC, 128, -1)], 2
        )
        return np.ascontiguousarray(
            Wab.transpose(1, 0, 2).reshape(128, -1)
        ).astype(bfd)

    def pack1(W):
        return np.ascontiguousarray(
            W.T.reshape(NEC, 128, -1).transpose(1, 0, 2).reshape(128, -1)
        ).astype(bfd)

    chot = np.zeros((128, HOT_W), np.float32)

    def setreg(name, arr):
        r, o, c = _HOT[name]
        chot[0:r, o : o + c] = arr

    setreg("wkv", pack2(Wk, Wv))
    setreg("wq", pack1(Wq))
    setreg("ident", consts["ident"])
    setreg("mloc4", consts["mloc4"])
    setreg("ce4", consts["ce4"])
    setreg("co4", consts["co4"])

    cst = np.zeros((128, CST_W), np.float32)
    cst[0:K, 0] = bk
    cst[K : 2 * K, 0] = bv
    cst[0:K, 1] = bq
    cst[:, 3] = np.concatenate([consts["wge"]] * (128 // C))
    cst[:, 4] = np.concatenate([consts["wgo"]] * (128 // C))

    shared = {
        "chot": chot.astype(bfd),
        "woT": np.ascontiguousarray(Wo.T).astype(bfd),
    }

    def pack_x(xh):
        # [E, HT] -> [NG*2*128, NEC*HW2]: per half-group, the exact SBUF
        # tile region as one contiguous block
        v = xh.reshape(NEC, 128, NG, 2, HW2).transpose(2, 3, 1, 0, 4)
        return np.ascontiguousarray(v.reshape(NG * 2 * 128, NEC * HW2))

    def pack_pre(xh):
        # last PRE prefix rows -> [128, NEC*PRE] contiguous block
        v = xh[:, HT - PRE :].reshape(NEC, 128, PRE).transpose(1, 0, 2)
        return np.ascontiguousarray(v.reshape(128, NEC * PRE))

    zeros_pre = np.zeros((128, NEC * PRE), bfd)
    in_maps = []
    for c in range(N_CORES):
        b, h = c // 2, c % 2
        xbT = np.ascontiguousarray(x[b].T).astype(bfd)  # [E, T]
        m = dict(shared)
        cstc = cst.copy()
        cstc[:, 2] = float(h)
        m["cst"] = cstc
        m["xpre2"] = pack_pre(xbT[:, :HT]) if h == 1 else zeros_pre
        m["xq2"] = pack_x(xbT[:, h * HT : (h + 1) * HT])
        in_maps.append(m)
    return in_maps


def run(inputs, trace=False):
    """Run on 8 cores; returns (output, BassKernelResults)."""
    nc = _get_program()
    in_maps = _make_in_maps(**{k: np.asarray(v) for k, v in inputs.items()})
    res = bass_utils.run_bass_kernel_spmd(
        nc, in_maps, core_ids=list(range(N_CORES)), trace=trace
    )
    out = np.empty((B, T, E), np.float32)
    for c in range(N_CORES):
        b, h = c // 2, c % 2
        out[b, h * HT : (h + 1) * HT, :] = res.results[c]["out"].astype(np.float32)
    return out, res


def kernel(**inputs):
    out, _ = run(inputs, trace=False)
    return out


# revision 18
# speedup vs baseline: 1.0589x; 1.0589x over previous
"""Trainium2 Bass kernel for nn_Decay2D (decay-masked linear attention).

Math: the reference's Hillis-Steele scan with decay-squaring order composes
to coefficient d^ceil((t-s)/2) on store[s] = scale*k_s v_s^T, so

    out[t] = scale^2 * sum_{s<=t} d^ceil((t-s)/2) (q_t . k_s) v_s  @ Wo^T

computed as chunked linear attention with two [K,V] carry states (even/odd
decay chains), never materializing the [B,T,K,V] memory.

Sharding: 8 cores = 4 batches x 2 sequence halves. Each core builds the
carry state over its (truncated) prefix rows and runs full attention +
output projection for its own 1024 rows.

Implementation notes (final, ~40.5us HW exec vs 56.6us baseline):
- bf16 on the PE, fp32 PSUM accumulation, fp32 carry states, bf16 output
  (converted back to fp32 on host; adds ~0.03% fro error, gate is 2e-2).
- Biases folded into activation bias= APs (no bias matmuls / ones row);
  k|v transposed jointly (one [128,128] PE transpose per chunk); parity
  decay weights applied as strided tensor_scalar ops on the k columns.
- Dual DMA queues: x loads on the sync HWDGE queue, constants on the
  scalar HWDGE queue (aggregate ~530GB/s); the constant pack is split so
  the projection weights land first and mask constants trail.
- Group-0 projections run per half-group (N=256 matmuls) so the PE
  starts right after the first 0.5MB of x lands.
- Interleaved schedule tuned against the trace: the sequential DVE carry
  chain and mask muls are slotted where PE has independent work, and the
  attention/out-projection pipeline keeps the 3-buffer po PSUM rotation
  fed (out copies split vector/scalar; final chunk DMA split across both
  HWDGE queues).
- gpsimd is avoided for tensor ops (cannot access PSUM, ~10x slower).
"""

from contextlib import ExitStack

import numpy as np

import concourse.bass as bass
import concourse.bacc as bacc
import concourse.mybir as mybir
import concourse.tile as tile
from concourse import bass_utils
from concourse.alu_op_type import AluOpType
from concourse.bass import ts

F32 = mybir.dt.float32
BF16 = mybir.dt.bfloat16
SIG = mybir.ActivationFunctionType.Sigmoid
IDENT = mybir.ActivationFunctionType.Identity

B, T, E, K, V = 4, 2048, 1024, 64, 64
DECAY = 0.9
C = 128          # chunk length
HT = T // 2      # rows per core (sequence half)
NCH = HT // C    # chunks per half (8)
NEC = E // 128   # embed sub-chunks (8)
GW = 512         # group width: 4 chunks per PSUM bank
GCH = GW // C    # chunks per group (4)
NG = HT // GW    # groups per half (2)
DC2 = float(DECAY ** (C // 2))
N_CORES = 8
HW2 = GW // 2    # half-group width
PRE = 128        # truncated prefix length (1 chunk; older rows decay < 2e-3)

# packed-constants layout (single bf16 pack)
def _mklayout(regions):
    out, off = {}, 0
    for n, r, c in regions:
        out[n] = (r, off, c)
        off += c
    return out, off


_HOT, HOT_W = _mklayout([
    ("wkv", 128, NEC * 2 * K), ("wq", 128, NEC * K), ("ident", 128, 128),
    ("mloc4", 128, GW), ("ce4", K, GW), ("co4", K, GW),
])
# f32 sidecar columns: bkv, bq, gamma, wge, wgo
CST_W = 5


def _host_constants():
    d = DECAY
    scale2 = 1.0 - d
    i = np.arange(C)
    j = np.arange(C)
    delta = i[:, None] - j[None, :]
    # intra-chunk decay mask, transposed to [tcol(j=src), trow(i=dst)],
    # scale^2 folded
    mloc = np.where(delta >= 0, d ** np.ceil(delta / 2.0), 0.0) * scale2
    mloc4 = np.tile(np.ascontiguousarray(mloc.T), (1, GCH)).astype(np.float32)
    # boundary coefficient per local row i (scale^2 folded), split by parity
    c = d ** np.ceil((i + 1) / 2.0) * scale2
    ce = np.where(i % 2 == 0, c, 0.0).astype(np.float32)
    co = np.where(i % 2 == 1, c, 0.0).astype(np.float32)
    ce4 = np.tile(np.broadcast_to(ce, (K, C)), (1, GCH)).astype(np.float32)
    co4 = np.tile(np.broadcast_to(co, (K, C)), (1, GCH)).astype(np.float32)
    # state-update row weights (per t within chunk)
    u_o = np.where(j % 2 == 1, d ** ((C - 1 - j) / 2.0), 0.0)
    u_e = np.where(j % 2 == 0, d ** ((C - 2 - j) / 2.0), 0.0)
    wge = (u_o + u_e).astype(np.float32)
    wgo = (u_o + d * u_e).astype(np.float32)
    return {
        "mloc4": mloc4,
        "ce4": np.ascontiguousarray(ce4),
        "co4": np.ascontiguousarray(co4),
        "wge": wge,
        "wgo": wgo,
        "ident": np.eye(128, dtype=np.float32),
    }


def _build_program():
    nc = bacc.Bacc(
        "TRN2",
        debug=False,
        enable_asserts=False,
        target_bir_lowering=False,
        num_devices=N_CORES,
    )

    def din(name, shape, dtype=F32):
        return nc.dram_tensor(name, shape, dtype, kind="ExternalInput").ap()

    chot_d = din("chot", [128, HOT_W], BF16)
    cst_d = din("cst", [128, CST_W], F32)
    xq2 = din("xq2", [NG * 2 * 128, NEC * HW2], BF16)
    xpre2 = din("xpre2", [128, NEC * PRE], BF16)
    woT_d = din("woT", [V, E], BF16)
    out_d = nc.dram_tensor("out", [HT, E], BF16, kind="ExternalOutput").ap()

    with ExitStack() as ctx:
        tc = ctx.enter_context(tile.TileContext(nc))

        consts = ctx.enter_context(tc.tile_pool(name="consts", bufs=1))
        state = ctx.enter_context(tc.tile_pool(name="state", bufs=1))
        xpool = ctx.enter_context(tc.tile_pool(name="xg", bufs=4))
        spool = ctx.enter_context(tc.tile_pool(name="sml", bufs=2))
        opool = ctx.enter_context(tc.tile_pool(name="osb", bufs=6))
        pbig = ctx.enter_context(tc.tile_pool(name="pbig", bufs=3, space="PSUM"))
        pscore = ctx.enter_context(tc.tile_pool(name="pscore", bufs=1, space="PSUM"))
        psml = ctx.enter_context(tc.tile_pool(name="psml", bufs=2, space="PSUM"))
        pattn = ctx.enter_context(tc.tile_pool(name="pattn", bufs=2, space="PSUM"))

        # ---- loads: x on sync queue, constants on scalar queue ----
        def load_xgroup(g, name):
            xg = xpool.tile([128, NEC * GW], BF16, tag="xg", name=name)
            for hh in range(2):
                idx = g * 2 + hh
                nc.sync.dma_start(
                    xg[:, ts(hh, NEC * HW2)],
                    xq2[idx * 128 : (idx + 1) * 128, :],
                )
            return xg

        xg0 = load_xgroup(0, "xg0")
        xp = xpool.tile([128, NEC * PRE], BF16, tag="xp", name="xp", bufs=1)
        nc.sync.dma_start(xp[:], xpre2[:])
        xg1 = load_xgroup(1, "xg1")

        W_SPLIT = _HOT["mloc4"][1]   # weights | masks boundary
        chot = consts.tile([128, HOT_W], BF16, name="chot")
        nc.scalar.dma_start(chot[:, 0:W_SPLIT], chot_d[:, 0:W_SPLIT])
        cst = consts.tile([128, CST_W], F32, name="cst")
        nc.scalar.dma_start(cst[:], cst_d[:])
        wo = consts.tile([V, E], BF16, name="wo")
        nc.scalar.dma_start(wo[:], woT_d[:])
        nc.scalar.dma_start(chot[:, W_SPLIT:], chot_d[:, W_SPLIT:])

        def reg(name):
            r, o, c = _HOT[name]
            return chot[0:r, o : o + c]

        wkv, wq, ident = reg("wkv"), reg("wq"), reg("ident")
        mloc4, ce4, co4 = reg("mloc4"), reg("ce4"), reg("co4")
        bq = cst[0:K, 1:2]
        gamma = cst[:, 2:3]
        wge = cst[:, 3:4]
        wgo = cst[:, 4:5]

        def xg_rhs(xg, ec):
            return xg.rearrange("p (hh a t) -> p a hh t", hh=2, a=NEC)[:, ec]

        qT_all = consts.tile([K, HT], BF16, name="qT_all")
        lt_all = consts.tile([V, HT], BF16, name="lt_all")
        geo_all = state.tile([2 * K, NCH * V], F32, name="geo_all")
        geo_bf = state.tile([2 * K, NCH * V], BF16, name="geo_bf")
        kvTs, knvs, kgeos = {}, {}, {}
        pus, ps_l, sT_l, qTeo_l = {}, {}, {}, {}

        # ---------- building blocks ----------
        def proj_group(g, xg, halves=False, kv_hook=None):
            pkv = pbig.tile([2 * K, GW], F32, tag="pB", name=f"pkv_g{g}")
            if halves:
                # per-half matmuls: compute starts after the first 0.5MB half
                # of the x group lands instead of the full 1MB
                xv = xg.rearrange("p (hh a t) -> p a hh t", hh=2, a=NEC)
                for hh in range(2):
                    for ec in range(NEC):
                        nc.tensor.matmul(pkv[:, ts(hh, HW2)],
                                         wkv[:, ts(ec, 2 * K)], xv[:, ec, hh],
                                         start=(ec == 0), stop=(ec == NEC - 1))
            else:
                for ec in range(NEC):
                    nc.tensor.matmul(pkv[:], wkv[:, ts(ec, 2 * K)],
                                     xg_rhs(xg, ec),
                                     start=(ec == 0), stop=(ec == NEC - 1))
            if kv_hook is not None:
                kv_hook()
            kvT = spool.tile([2 * K, GW], BF16, tag=f"kvT_g{g}", name="kvT")
            nc.scalar.activation(kvT[0:K, :], pkv[0:K, :], SIG, bias=cst[0:K, 0:1])
            nc.scalar.activation(kvT[K : 2 * K, :], pkv[K : 2 * K, :], IDENT,
                                 bias=cst[K : 2 * K, 0:1])
            pg = pbig.tile([K, GW], F32, tag="pB", name=f"pq_g{g}")
            for ec in range(NEC):
                nc.tensor.matmul(pg[:], wq[:, ts(ec, K)], xg_rhs(xg, ec),
                                 start=(ec == 0), stop=(ec == NEC - 1))
            nc.scalar.activation(qT_all[:, ts(g, GW)], pg[:], SIG, bias=bq)
            kvTs[g] = kvT

        def trans_group(g):
            kvT = kvTs[g]
            pknv = psml.tile([C, GCH * 2 * K], BF16, tag="pS", name=f"pknv_g{g}")
            for cl in range(GCH):
                nc.tensor.matmul(pknv[:, ts(cl, 2 * K)], kvT[:, ts(cl, C)],
                                 ident[:], is_transpose=True)
            knv = spool.tile([C, GCH * 2 * K], BF16, tag=f"knv_g{g}", name="knv")
            nc.vector.tensor_copy(knv[:], pknv[:])
            kgeo = spool.tile([C, GCH * 2 * K], BF16, tag=f"kg_g{g}", name="kgeo")
            kview = knv.rearrange("p (cl two k) -> p cl two k", cl=GCH, two=2)
            gview = kgeo.rearrange("p (cl two k) -> p cl two k", cl=GCH, two=2)
            nc.vector.tensor_scalar_mul(gview[:, :, 0], kview[:, :, 0], wge)
            nc.vector.tensor_scalar_mul(gview[:, :, 1], kview[:, :, 0], wgo)
            knvs[g], kgeos[g] = knv, kgeo

        def vview(knv, cl):
            return knv[:, cl * 2 * K + K : (cl + 1) * 2 * K]

        def pu_group(g):
            pu = psml.tile([2 * K, GCH * V], F32, tag="pS", name=f"pu_g{g}")
            for cl in range(GCH):
                nc.tensor.matmul(pu[:, ts(cl, V)], kgeos[g][:, ts(cl, 2 * K)],
                                 vview(knvs[g], cl), start=True, stop=True)
            pus[g] = pu

        def chain_seg(g):
            # carry chain states i in [g*GCH+1 .. g*GCH+GCH] using pu_g
            for i in range(g * GCH + 1, g * GCH + GCH + 1):
                if i >= NCH:
                    break
                cl = (i - 1) % GCH
                nc.vector.scalar_tensor_tensor(
                    geo_all[:, ts(i, V)], geo_all[:, ts(i - 1, V)], DC2,
                    pus[g][:, ts(cl, V)], AluOpType.mult, AluOpType.add,
                )
            lo, hi = g * GCH, min(g * GCH + GCH, NCH)
            nc.scalar.copy(geo_bf[:, lo * V : hi * V], geo_all[:, lo * V : hi * V])

        def scores_group(g):
            psc = pscore.tile([C, GW], F32, tag="pSc", name=f"ps_g{g}")
            for cl in range(GCH):
                i = g * GCH + cl
                nc.tensor.matmul(psc[:, ts(cl, C)], kvTs[g][0:K, ts(cl, C)],
                                 qT_all[:, ts(i, C)], start=True, stop=True)
            ps_l[g] = psc

        def masks_group(g):
            sT_b = spool.tile([C, GW], BF16, tag=f"sm{g}", name="sT_b")
            nc.vector.tensor_mul(sT_b[:], ps_l[g][:], mloc4[:])
            sT_l[g] = sT_b
            qTeo = spool.tile([2 * K, GW], BF16, tag=f"qeo{g}", name="qTeo")
            nc.vector.tensor_mul(qTeo[0:K, :], qT_all[:, ts(g, GW)], ce4[:])
            nc.vector.tensor_mul(qTeo[K : 2 * K, :], qT_all[:, ts(g, GW)], co4[:])
            qTeo_l[g] = qTeo

        def attn_chunk(i):
            g, cl = i // GCH, i % GCH
            plt = pattn.tile([V, C], F32, tag="pA", name=f"plt_{i}")
            nc.tensor.matmul(plt[:], vview(knvs[g], cl), sT_l[g][:, ts(cl, C)],
                             start=True, stop=False)
            nc.tensor.matmul(plt[:], geo_bf[:, ts(i, V)], qTeo_l[g][:, ts(cl, C)],
                             start=False, stop=True)
            nc.vector.tensor_copy(lt_all[:, ts(i, C)], plt[:])

        def outproj_chunk(i, split_dma=False):
            out_sb = opool.tile([C, E], BF16, tag="osb", name=f"out_sb_{i}")
            for h in range(2):
                po = pbig.tile([C, GW], F32, tag="pB", name=f"po_{i}_{h}")
                nc.tensor.matmul(po[:], lt_all[:, ts(i, C)], wo[:, ts(h, GW)],
                                 start=True, stop=True)
                if h == 0 and i >= 2:
                    nc.vector.tensor_copy(out_sb[:, ts(h, GW)], po[:])
                else:
                    nc.scalar.copy(out_sb[:, ts(h, GW)], po[:])
                if split_dma:
                    eng = nc.sync if h == 0 else nc.scalar
                    eng.dma_start(out_d[ts(i, C), ts(h, GW)],
                                  out_sb[:, ts(h, GW)])
            if not split_dma:
                nc.sync.dma_start(out_d[ts(i, C), :], out_sb[:])

        # ---------- schedule ----------
        # group 0 projections + transposes
        proj_group(0, xg0, halves=True)
        trans_group(0)

        # prefix phase-1 projection (PE) while scalar runs g0 sigmoids
        pkv1 = pbig.tile([2 * K, PRE], F32, tag="pB", name="pkv1")
        for ec in range(NEC):
            nc.tensor.matmul(pkv1[:], wkv[:, ts(ec, 2 * K)], xp[:, ts(ec, PRE)],
                             start=(ec == 0), stop=(ec == NEC - 1))
        kvT1 = spool.tile([2 * K, PRE], BF16, tag="kvT1", name="kvT1")
        nc.scalar.activation(kvT1[0:K, :], pkv1[0:K, :], SIG, bias=cst[0:K, 0:1])
        nc.scalar.activation(kvT1[K : 2 * K, :], pkv1[K : 2 * K, :], IDENT,
                             bias=cst[K : 2 * K, 0:1])

        # scores g0 (PE busy while kvT1 sigmoid completes)
        scores_group(0)

        # prefix transpose + state
        pknv1 = psml.tile([C, 2 * K], BF16, tag="pS", name="pknv1")
        nc.tensor.matmul(pknv1[:], kvT1[:], ident[:], is_transpose=True)
        knv1 = spool.tile([C, 2 * K], BF16, tag="knv1", name="knv1")
        nc.vector.tensor_copy(knv1[:], pknv1[:])
        kgeo1 = spool.tile([C, 2 * K], BF16, tag="kg1", name="kgeo1")
        nc.vector.tensor_scalar_mul(kgeo1[:, 0:K], knv1[:, 0:K], wge)
        nc.vector.tensor_scalar_mul(kgeo1[:, K : 2 * K], knv1[:, 0:K], wgo)
        pu1 = pattn.tile([2 * K, V], F32, tag="pA", name="pu1")
        nc.tensor.matmul(pu1[:], kgeo1[:], vview(knv1, 0), start=True, stop=True)

        # g0 states + chain segment A (DVE) + masks g0
        pu_group(0)
        nc.vector.tensor_scalar_mul(geo_all[:, 0:V], pu1[:], gamma)
        chain_seg(0)
        masks_group(0)

        # group 1 projections + transposes (PE busy under DVE chain/masks);
        # attn chunks 0-1 interleave into the projection stream to spread
        # PE load out of the throttled attention window
        def _hook():
            attn_chunk(0)
            attn_chunk(1)
        proj_group(1, xg1, kv_hook=_hook)
        outproj_chunk(0)
        trans_group(1)
        outproj_chunk(1)
        pu_group(1)
        chain_seg(1)
        scores_group(1)
        masks_group(1)
        attn_chunk(2)
        outproj_chunk(2)
        attn_chunk(3)
        attn_chunk(4)
        outproj_chunk(3)
        for i in range(5, NCH):
            attn_chunk(i)
            outproj_chunk(i - 1)
        outproj_chunk(NCH - 1, split_dma=True)

    nc.compile()
    return nc


_CACHE = {}


def _get_program():
    if "nc" not in _CACHE:
        _CACHE["nc"] = _build_program()
    return _CACHE["nc"]


def _make_in_maps(x, Wk, bk, Wv, bv, Wq, bq, Wo):
    import ml_dtypes

    bfd = ml_dtypes.bfloat16
    consts = _host_constants()

    def pack2(Wa, Wb):
        # [128, NEC*(outA+outB)]: per embed sub-chunk, [Wa_ec | Wb_ec] columns
        Wab = np.concatenate(
            [Wa.T.reshape(NEC, 128, -1), Wb.T.reshape(NEC, 128, -1)], 2
        )
        return np.ascontiguousarray(
            Wab.transpose(1, 0, 2).reshape(128, -1)
        ).astype(bfd)

    def pack1(W):
        return np.ascontiguousarray(
            W.T.reshape(NEC, 128, -1).transpose(1, 0, 2).reshape(128, -1)
        ).astype(bfd)

    chot = np.zeros((128, HOT_W), np.float32)

    def setreg(name, arr):
        r, o, c = _HOT[name]
        chot[0:r, o : o + c] = arr

    setreg("wkv", pack2(Wk, Wv))
    setreg("wq", pack1(Wq))
    setreg("ident", consts["ident"])
    setreg("mloc4", consts["mloc4"])
    setreg("ce4", consts["ce4"])
    setreg("co4", consts["co4"])

    cst = np.zeros((128, CST_W), np.float32)
    cst[0:K, 0] = bk
    cst[K : 2 * K, 0] = bv
    cst[0:K, 1] = bq
    cst[:, 3] = np.concatenate([consts["wge"]] * (128 // C))
    cst[:, 4] = np.concatenate([consts["wgo"]] * (128 // C))

    shared = {
        "chot": chot.astype(bfd),
        "woT": np.ascontiguousarray(Wo.T).astype(bfd),
    }

    def pack_x(xh):
        # [E, HT] -> [NG*2*128, NEC*HW2]: per half-group, the exact SBUF
        # tile region as one contiguous block
        v = xh.reshape(NEC, 128, NG, 2, HW2).transpose(2, 3, 1, 0, 4)
        return np.ascontiguousarray(v.reshape(NG * 2 * 128, NEC * HW2))

    def pack_pre(xh):
        # last PRE prefix rows -> [128, NEC*PRE] contiguous block
        v = xh[:, HT - PRE :].reshape(NEC, 128, PRE).transpose(1, 0, 2)
        return np.ascontiguousarray(v.reshape(128, NEC * PRE))

    zeros_pre = np.zeros((128, NEC * PRE), bfd)
    in_maps = []
    for c in range(N_CORES):
        b, h = c // 2, c % 2
        xbT = np.ascontiguousarray(x[b].T).astype(bfd)  # [E, T]
        m = dict(shared)
        cstc = cst.copy()
        cstc[:, 2] = float(h)
        m["cst"] = cstc
        m["xpre2"] = pack_pre(xbT[:, :HT]) if h == 1 else zeros_pre
        m["xq2"] = pack_x(xbT[:, h * HT : (h + 1) * HT])
        in_maps.append(m)
    return in_maps


def run(inputs, trace=False):
    """Run on 8 cores; returns (output, BassKernelResults)."""
    nc = _get_program()
    in_maps = _make_in_maps(**{k: np.asarray(v) for k, v in inputs.items()})
    res = bass_utils.run_bass_kernel_spmd(
        nc, in_maps, core_ids=list(range(N_CORES)), trace=trace
    )
    out = np.empty((B, T, E), np.float32)
    for c in range(N_CORES):
        b, h = c // 2, c % 2
        out[b, h * HT : (h + 1) * HT, :] = res.results[c]["out"].astype(np.float32)
    return out, res


def kernel(**inputs):
    out, _ = run(inputs, trace=False)
    return out
